# revision 19
# baseline (speedup 1.0000x reference)
"""MQA attention (B=2, Lq=Lkv=2048, F=1024, H=16, D=64) on 8 TRN2 cores.

Sharding: core = (batch, query-block-of-512). Each core computes its full
output rows (all 16 heads + output projection) -> no collectives; host
concatenates per-core yT slabs.

v4 dataflow (fp16 operands everywhere off-PSUM, fp32 PSUM accumulate):
  - All DMA'd tensors are host-prepped into p-major contiguous layouts so
    every dma_start is a simple 2D transfer (strided rearranges cost
    6-16us of descriptor generation on the queue).
  - kv projection + k-RoPE first, then per pair j: q-projection+RoPE for
    j is emitted, then pair j's attention - so the PE starts scoring
    ~30us earlier and later q-projections hide in the ACT-bound
    attention pipeline.
  - S^T per head-pair = two CONCURRENT K=64 row-tiled matmuls
    (tile_position (0,0)/(64,0)); P^T = exp on ACT [128,1024] supertiles;
    ONE masked multiply per chunk on DVE (mask broadcast over the 2 heads
    via 0-stride AP); O_aug^T accumulates via V_aug (ones column -> softmax
    denominator in row 64).
  - Deferred normalization: unnormalized O + den evacuate per pair (one
    DVE copy + DMAs); denominators partition-scatter into 32-aligned
    4-row batches, reciprocal_approx_fast per 2 pairs, broadcast via K=1
    matmuls from a partition-0 row, one in-place multiply per pair.
  - Output projection accumulates j-inner on 'acc'-tag PSUM slots.
"""

import numpy as np

import concourse.bass as bass
import concourse.tile as tile
from concourse import bacc, mybir
from concourse import bass_utils
from concourse.bass import ts
from concourse.masks import make_identity

F32 = mybir.dt.float32
F16 = mybir.dt.float16

B, L, F, H, D = 2, 2048, 1024, 16, 64
LQ = 512            # query rows per core
LK = 2048           # kv rows (full)
NCORES = 8
PAIRS = H // 2      # head pairs (one qT partition block each)
FCH = F // 128      # f contraction chunks
KCH = LK // 128     # lk chunks
NL = LK // LQ       # kv column blocks

_CACHED = {}


def build_nc():
    nc = bacc.Bacc("TRN2", target_bir_lowering=False, debug=False,
                   num_devices=NCORES)
    dt_in = [
        ("xq_t", [128, FCH, LQ], F16),         # [p, f, lq]
        ("xkv_t", [NL, 128, FCH, LQ], F16),    # [l, p, f, lq]
        ("mask_t", [128, KCH, LQ], F16),       # [p, c, lq]
        ("wq", [128, FCH, FCH, 128], F16),     # [p, j, f, m]
        ("wkv", [128, FCH, 128], F16),         # [p, f, m]
        ("wo", [128, FCH, FCH, 128], F16),     # [p, fb, j, m]
        ("bqbo", [128, 2 * FCH], F32),         # cols 0:8 bq-blocks, 8:16 bo
        ("bkv", [2 * D], F32),
        ("cosq", [128, LQ], F16),
        ("sinq", [128, LQ], F16),
        ("cksk", [D, 2 * LK], F16),            # [p, (cos|sin)*lk]
    ]
    t = {name: nc.dram_tensor(name, shape, dt, kind="ExternalInput")
         for name, shape, dt in dt_in}
    yT = nc.dram_tensor("yT", [F, LQ], F32, kind="ExternalOutput")

    with tile.TileContext(nc) as tc:
        with (
            tc.tile_pool(name="persist", bufs=1) as persist,
            tc.tile_pool(name="ptiles", bufs=4) as ptp,
            tc.tile_pool(name="small", bufs=4) as small,
            tc.tile_pool(name="stgp", bufs=4) as stgp,
            tc.tile_pool(name="rbsp", bufs=4) as rbsp,
            tc.tile_pool(name="qrp", bufs=2) as qrp,
            tc.tile_pool(name="rtp", bufs=3) as rtp,
            tc.tile_pool(name="psacc", bufs=4, space="PSUM") as psacc,
            tc.tile_pool(name="psst", bufs=2, space="PSUM") as psst,
        ):
            qrot = persist.tile([128, PAIRS, LQ], F16)    # rotated qT
            ktops = persist.tile([128, LK], F16)  # kT in rows 0:64 AND 64:128
            vaug = persist.tile([128, KCH, D + 1], F16)   # V chunks + ones col
            obig = persist.tile([128, PAIRS, LQ], F16)    # O^T (unnorm->norm)
            # denominators partition-scattered: pairs 0..5 -> rows 0..11,
            # pairs 6..7 -> rows 32..35 (32-aligned engine bases for the two
            # batched reciprocals)
            densp = persist.tile([36, LQ], F16)
            rec32 = persist.tile([36, LQ], F32)
            recrow = persist.tile([1, 2 * PAIRS, LQ], F32)  # partition 0
            onesr = persist.tile([1, 128], F32)
            nc.vector.memset(onesr, 1.0)
            mt = persist.tile([128, KCH, LQ], F16)        # maskT resident

            # ---- small consts first on the sync queue ----
            bqbo = small.tile([128, 2 * FCH], F32, tag="bias")
            nc.sync.dma_start(bqbo, t["bqbo"].ap())
            bq_sb = bqbo[:, 0:FCH]
            bo_sb = bqbo[:, FCH:2 * FCH]
            bkv_sb = small.tile([128, 1], F32, tag="bias2")
            nc.sync.dma_start(bkv_sb, t["bkv"].ap().unsqueeze(1))
            cq = persist.tile([128, LQ], F16)
            sq = persist.tile([128, LQ], F16)
            cksk = persist.tile([D, 2, LK], F16)
            nc.sync.dma_start(cq, t["cosq"].ap())
            nc.sync.dma_start(sq, t["sinq"].ap())
            nc.sync.dma_start(cksk,
                              t["cksk"].ap().rearrange("p (a l) -> p a l", a=2))
            ck = cksk[:, 0, :]
            sk = cksk[:, 1, :]

            # ---- inputs split across the sync and gpsimd DMA queues in
            # dependency order (each queue sustains ~100 GB/s; two run in
            # parallel). kv chain on sync; q-side + first mask half on
            # gpsimd; late consumers (mask tail, wo) at the back. ----
            wkv_sb = persist.tile([128, FCH, 128], F16)
            nc.sync.dma_start(wkv_sb, t["wkv"].ap())
            xkvt = []
            for l in range(NL):
                xkv = persist.tile([128, FCH, LQ], F16, tag=f"xkv{l}")
                nc.sync.dma_start(xkv, t["xkv_t"].ap()[l])
                xkvt.append(xkv)
            xq = persist.tile([128, FCH, LQ], F16)
            nc.gpsimd.dma_start(xq, t["xq_t"].ap())
            wqall = persist.tile([128, FCH, FCH, 128], F16)
            nc.gpsimd.dma_start(wqall, t["wq"].ap())
            nc.gpsimd.dma_start(mt[:, 0:KCH // 2, :],
                                t["mask_t"].ap()[:, 0:KCH // 2, :])
            nc.sync.dma_start(mt[:, KCH // 2:KCH, :],
                              t["mask_t"].ap()[:, KCH // 2:KCH, :])
            woall = persist.tile([128, FCH, FCH, 128], F16)
            nc.sync.dma_start(woall, t["wo"].ap())

            idtf = small.tile([128, 128], F32, tag="identf")
            make_identity(nc, idtf)
            idt = small.tile([128, 128], F16, tag="ident")
            nc.vector.tensor_copy(idt, idtf)
            # halves-swap permutation matrix: M[p, p-xor-32-within-head] = 1
            swpf = small.tile([128, 128], F32, tag="swpf")
            nc.gpsimd.memset(swpf, 0.0)
            for o1, o2 in ((0, 32), (32, 0), (64, 96), (96, 64)):
                nc.gpsimd.affine_select(
                    out=swpf[o1:o1 + 32, o2:o2 + 32],
                    in_=swpf[o1:o1 + 32, o2:o2 + 32],
                    compare_op=mybir.AluOpType.not_equal, fill=1.0,
                    base=0, pattern=[[-1, 32]], channel_multiplier=1)
            swp = small.tile([128, 128], F16, tag="swp")
            nc.vector.tensor_copy(swp, swpf)

            # ======== phase A: kv projection + k-RoPE ========
            kvraw = persist.tile([128, LK], F16)
            with tc.tile_pool(name="ktmp", bufs=1) as ktp:
                for l in range(NL):
                    pkv = psacc.tile([128, LQ], F32, tag="acc")
                    for f in range(FCH):
                        nc.tensor.matmul(pkv, wkv_sb[:, f, :],
                                         xkvt[l][:, f, :],
                                         start=(f == 0), stop=(f == FCH - 1))
                    nc.vector.tensor_scalar_add(kvraw[:, ts(l, LQ)], pkv,
                                                bkv_sb[:, 0:1])

                tmk = ktp.tile([D, LK], F16, tag="ksin")
                nc.vector.tensor_mul(tmk, kvraw[0:64], sk)
                kc = ktp.tile([D, LK], F16, tag="kcos")
                nc.vector.tensor_mul(kc, kvraw[0:64], ck)
                for l in range(NL):
                    pswk = psacc.tile([128, LQ], F32, tag="acc")
                    nc.tensor.matmul(pswk[0:64], swp[0:64, 0:64],
                                     tmk[:, ts(l, LQ)], start=True, stop=True)
                    nc.vector.tensor_add(ktops[0:64, ts(l, LQ)],
                                         kc[:, ts(l, LQ)], pswk[0:64])
                nc.gpsimd.dma_start(ktops[64:128], ktops[0:64])

            def qproj_rope(j):
                psq = psacc.tile([128, LQ], F32, tag="acc")
                for f in range(FCH):
                    nc.tensor.matmul(psq, wqall[:, j, f, :], xq[:, f, :],
                                     start=(f == 0), stop=(f == FCH - 1))
                qraw = qrp.tile([128, LQ], F16, tag="qraw")
                nc.vector.tensor_scalar_add(qraw, psq, bq_sb[:, j:j + 1])
                tmq = rtp.tile([128, LQ], F16, tag="qsin")
                nc.vector.tensor_mul(tmq, qraw, sq)
                psw = psacc.tile([128, LQ], F32, tag="acc")
                nc.tensor.matmul(psw, swp, tmq, start=True, stop=True)
                qc = rtp.tile([128, LQ], F16, tag="qcos")
                nc.vector.tensor_mul(qc, qraw, cq)
                nc.vector.tensor_add(qrot[:, j, :], qc, psw)

            # ========== phase C: attention (q-projection interleaved) =======
            for j in range(PAIRS):
                qproj_rope(j)
                if j == 0:
                    # V_aug: transpose v chunks, append ones column
                    nc.vector.memset(vaug[:, :, D:D + 1], 1.0)
                    for c in range(KCH):
                        tp = psacc.tile([128, D], F16, tag="acc")
                        nc.tensor.transpose(tp, kvraw[64:128, ts(c, 128)],
                                            idt[64:128, 64:128])
                        nc.vector.tensor_copy(vaug[:, c, 0:D], tp)

                oa = psacc.tile([128, LQ], F32, tag="acc")
                ob = psacc.tile([128, LQ], F32, tag="acc")
                for c in range(KCH):
                    if j == PAIRS - 2 and c == 5:
                        # batched reciprocal for pairs 0..5, placed mid-pair
                        # so the DVE-stream bubble is absorbed by the chunk
                        # pipeline's slack instead of stalling the PE
                        nc.vector.reciprocal(rec32[0:12, :], densp[0:12, :])
                        nc.sync.dma_start(recrow[0:1, 0:12, :], rec32[0:12, :])
                    st = psst.tile([128, 2, LQ], F32, tag="st")
                    # two concurrent K=64 row-tiled matmuls (heads 2j, 2j+1)
                    nc.tensor.matmul(st[:, 0, :], ktops[0:64, ts(c, 128)],
                                     qrot[0:64, j, :], start=True, stop=True)
                    nc.tensor.matmul(st[:, 1, :], ktops[64:128, ts(c, 128)],
                                     qrot[64:128, j, :], start=True, stop=True)
                    pt = ptp.tile([128, 2, LQ], F16, tag="p")
                    nc.scalar.activation(pt, st,
                                         mybir.ActivationFunctionType.Exp)
                    # one masked multiply for both heads (mask broadcast)
                    mb = mt[:, c, :].unsqueeze(1).broadcast_to((128, 2, LQ))
                    nc.vector.tensor_mul(pt, pt, mb)
                    nc.tensor.matmul(oa[0:D + 1, :], vaug[:, c, :],
                                     pt[:, 0, :], start=(c == 0),
                                     stop=(c == KCH - 1))
                    nc.tensor.matmul(ob[0:D + 1, :], vaug[:, c, :],
                                     pt[:, 1, :], start=(c == 0),
                                     stop=(c == KCH - 1))
                # evacuate unnormalized O^T + denominator row
                astg = stgp.tile([128, LQ], F16, tag="stg")
                nc.vector.tensor_copy(astg[0:D + 1], oa[0:D + 1])
                bstg = stgp.tile([128, LQ], F16, tag="stg")
                nc.vector.tensor_copy(bstg[0:D + 1], ob[0:D + 1])
                dp = 2 * j if j < PAIRS - 2 else 32 + 2 * (j - (PAIRS - 2))
                nc.sync.dma_start(obig[0:64, j, :], astg[0:64])
                nc.sync.dma_start(densp[dp:dp + 1, :], astg[64:65])
                nc.gpsimd.dma_start(obig[64:128, j, :], bstg[0:64])
                nc.gpsimd.dma_start(densp[dp + 1:dp + 2, :], bstg[64:65])

            # ======== normalization (all after the attention loop, so no
            # slow-dependency op ever precedes attention work in an engine's
            # static stream) ========
            def norm_pair(nj):
                ri = 2 * nj
                rbp = psacc.tile([128, LQ], F32, tag="acc")
                nc.tensor.matmul(rbp[0:64, :], onesr[0:1, 0:64],
                                 recrow[0:1, ri, :], start=True, stop=True)
                nc.tensor.matmul(rbp[64:128, :], onesr[0:1, 64:128],
                                 recrow[0:1, ri + 1, :], start=True, stop=True)
                rbs = rbsp.tile([128, LQ], F16, tag="rbs")
                nc.scalar.copy(rbs, rbp)
                nc.vector.tensor_mul(obig[:, nj, :], obig[:, nj, :], rbs)

            for nj in range(PAIRS - 2):
                norm_pair(nj)
            # pairs 6,7: reciprocal + gather, then normalize
            nc.vector.reciprocal(rec32[32:36, :], densp[32:36, :])
            nc.sync.dma_start(recrow[0:1, 12:16, :], rec32[32:36, :])
            norm_pair(PAIRS - 2)
            norm_pair(PAIRS - 1)

            # ================= phase D: output projection =================
            # two groups of 4 fb-chains; accumulate j=0..5 first so the
            # group's early matmuls overlap the pairs-6/7 normalization
            with tc.tile_pool(name="yout", bufs=2) as yout:
                for g in range(2):
                    fbs = range(4 * g, 4 * g + 4)
                    psys = {}
                    for fb in fbs:
                        psy_fb = psacc.tile([128, LQ], F32, tag="acc")
                        psys[fb] = psy_fb
                    for j in range(FCH):
                        for fb in fbs:
                            nc.tensor.matmul(psys[fb], woall[:, fb, j, :],
                                             obig[:, j, :],
                                             start=(j == 0),
                                             stop=(j == FCH - 1))
                    for fb in fbs:
                        ysb = yout.tile([128, LQ], F32, tag="y")
                        nc.vector.tensor_scalar_add(ysb, psys[fb],
                                                    bo_sb[:, fb:fb + 1])
                        eng = nc.sync if fb % 2 == 0 else nc.gpsimd
                        eng.dma_start(yT.ap()[ts(fb, 128), :], ysb)

    nc.compile()
    return nc


def _tables():
    """RoPE tables in halves-permuted basis: rows i (even-half) hold +sin,
    rows 32+i (odd-half) hold -sin (for the tmp-then-swap formulation)."""
    inv_freq = 1.0 / (10000.0 ** (np.arange(0, D, 2, dtype=np.float64) / D))
    ang = np.outer(inv_freq, np.arange(L, dtype=np.float64))  # [32, L]
    cos = np.cos(ang).astype(np.float32)
    sin = np.sin(ang).astype(np.float32)
    cos64 = np.concatenate([cos, cos], axis=0)                # [64, L]
    sin_sgn = np.concatenate([sin, -sin], axis=0)             # [64, L]
    return cos64, sin_sgn


def _pmajor(a, nch):
    """[nch*128, cols] -> [128, nch, cols] p-major contiguous."""
    return np.ascontiguousarray(
        a.reshape(nch, 128, -1).transpose(1, 0, 2)).astype(np.float16)


def _prep_weights(Wq, bq, Wk, bk, Wv, bv, Wo, bo):
    perm = np.concatenate([np.arange(0, D, 2), np.arange(1, D, 2)])
    WqP = np.asarray(Wq, dtype=np.float32)[:, :, perm].reshape(F, H * D)
    bqP = np.asarray(bq, dtype=np.float32)[:, perm].reshape(H * D)
    WkP = np.asarray(Wk, dtype=np.float32)[:, perm]
    bkP = np.asarray(bk, dtype=np.float32)[perm]
    Wkv = np.concatenate([WkP, np.asarray(Wv, dtype=np.float32)], axis=1)
    bkv = np.concatenate([bkP, np.asarray(bv, dtype=np.float32)])
    WoR = np.asarray(Wo, dtype=np.float32).reshape(H * D, F)
    bo_ = np.asarray(bo, dtype=np.float32)

    # wq[p, j, f, m] = WqP[f*128+p, j*128+m]
    wq_pret = np.ascontiguousarray(
        WqP.reshape(FCH, 128, FCH, 128).transpose(1, 2, 0, 3)).astype(
            np.float16)
    # wkv[p, f, m] = Wkv[f*128+p, m]
    wkv_pret = np.ascontiguousarray(
        Wkv.reshape(FCH, 128, 128).transpose(1, 0, 2)).astype(np.float16)
    # wo[p, fb, j, m] = WoR[j*128+p, fb*128+m]
    wo_pret = np.ascontiguousarray(
        WoR.reshape(FCH, 128, FCH, 128).transpose(1, 2, 0, 3)).astype(
            np.float16)
    bqbo = np.ascontiguousarray(np.concatenate(
        [bqP.reshape(FCH, 128).T, bo_.reshape(FCH, 128).T], axis=1))
    return wq_pret, wkv_pret, wo_pret, bqbo, bkv


def kernel(inputs_q, inputs_kv, mask, Wq, bq, Wk, bk, Wv, bv, Wo, bo):
    if "nc" not in _CACHED:
        _CACHED["nc"] = build_nc()
    nc = _CACHED["nc"]

    wq_pret, wkv_pret, wo_pret, bqbo, bkv = _prep_weights(
        Wq, bq, Wk, bk, Wv, bv, Wo, bo)

    cos64, sin_sgn = _tables()
    scale = 1.0 / np.sqrt(np.float32(D))
    cksk = np.ascontiguousarray(
        np.concatenate([cos64, sin_sgn], axis=1)).astype(np.float16)
    cosq_full = np.tile(cos64 * scale, (2, 1)).astype(np.float16)  # [128, L]
    sinq_full = np.tile(sin_sgn * scale, (2, 1)).astype(np.float16)

    xq = np.asarray(inputs_q, dtype=np.float32)
    xkv = np.asarray(inputs_kv, dtype=np.float32)
    mk = np.asarray(mask)

    in_maps = []
    for core in range(NCORES):
        b = core // 4
        qs = (core % 4) * LQ
        xq_t = _pmajor(xq[b, qs:qs + LQ, :].T, FCH)           # [128, f, lq]
        xkv_t = np.stack([
            _pmajor(xkv[b, l * LQ:(l + 1) * LQ, :].T, FCH)
            for l in range(NL)])                              # [l, 128, f, lq]
        mask_t = _pmajor(
            mk[b, 0, qs:qs + LQ, :].T.astype(np.float16), KCH)
        in_maps.append({
            "xq_t": xq_t,
            "xkv_t": xkv_t,
            "mask_t": mask_t,
            "wq": wq_pret,
            "wkv": wkv_pret,
            "wo": wo_pret,
            "bqbo": bqbo,
            "bkv": bkv,
            "cosq": np.ascontiguousarray(cosq_full[:, qs:qs + LQ]),
            "sinq": np.ascontiguousarray(sinq_full[:, qs:qs + LQ]),
            "cksk": cksk,
        })

    res = bass_utils.run_bass_kernel_spmd(nc, in_maps,
                                          core_ids=list(range(NCORES)))
    _CACHED["last_results"] = res
    _CACHED["last_maps"] = in_maps

    out = np.empty((B, L, F), dtype=np.float32)
    for core in range(NCORES):
        b = core // 4
        qs = (core % 4) * LQ
        out[b, qs:qs + LQ, :] = res.results[core]["yT"].T
    return out


# revision 21
# speedup vs baseline: 1.0462x; 1.0462x over previous
"""MQA attention (B=2, Lq=Lkv=2048, F=1024, H=16, D=64) on 8 TRN2 cores.

Sharding: core = (batch, query-block-of-512). Each core computes its full
output rows (all 16 heads + output projection) -> no collectives; host
concatenates per-core yT slabs.

v4 dataflow (fp16 operands everywhere off-PSUM, fp32 PSUM accumulate):
  - All DMA'd tensors are host-prepped into p-major contiguous layouts so
    every dma_start is a simple 2D transfer (strided rearranges cost
    6-16us of descriptor generation on the queue).
  - kv projection + k-RoPE first, then per pair j: q-projection+RoPE for
    j is emitted, then pair j's attention - so the PE starts scoring
    ~30us earlier and later q-projections hide in the ACT-bound
    attention pipeline.
  - S^T per head-pair = two CONCURRENT K=64 row-tiled matmuls
    (tile_position (0,0)/(64,0)); P^T = exp on ACT [128,1024] supertiles;
    ONE masked multiply per chunk on DVE (mask broadcast over the 2 heads
    via 0-stride AP); O_aug^T accumulates via V_aug (ones column -> softmax
    denominator in row 64).
  - Deferred normalization: unnormalized O + den evacuate per pair (one
    DVE copy + DMAs); denominators partition-scatter into 32-aligned
    4-row batches, reciprocal_approx_fast per 2 pairs, broadcast via K=1
    matmuls from a partition-0 row, one in-place multiply per pair.
  - Output projection accumulates j-inner on 'acc'-tag PSUM slots.
"""

import numpy as np

import concourse.bass as bass
import concourse.tile as tile
from concourse import bacc, mybir
from concourse import bass_utils
from concourse.bass import ts
from concourse.masks import make_identity

F32 = mybir.dt.float32
F16 = mybir.dt.float16

B, L, F, H, D = 2, 2048, 1024, 16, 64
LQ = 512            # query rows per core
LK = 2048           # kv rows (full)
NCORES = 8
PAIRS = H // 2      # head pairs (one qT partition block each)
FCH = F // 128      # f contraction chunks
KCH = LK // 128     # lk chunks
NL = LK // LQ       # kv column blocks

_CACHED = {}


def build_nc():
    nc = bacc.Bacc("TRN2", target_bir_lowering=False, debug=False,
                   num_devices=NCORES)
    dt_in = [
        ("xq_t", [128, FCH, LQ], F16),         # [p, f, lq]
        ("xkv_t", [NL, 128, FCH, LQ], F16),    # [l, p, f, lq]
        ("mask_t", [128, KCH, LQ], F16),       # [p, c, lq]
        ("wq", [128, FCH, FCH, 128], F16),     # [p, j, f, m]
        ("wkv", [128, FCH, 128], F16),         # [p, f, m]
        ("wo", [128, FCH, FCH, 128], F16),     # [p, fb, j, m]
        ("bqbo", [128, 2 * FCH], F32),         # cols 0:8 bq-blocks, 8:16 bo
        ("bkv", [2 * D], F32),
        ("cosq", [128, LQ], F16),
        ("sinq", [128, LQ], F16),
        ("cksk", [D, 2 * LK], F16),            # [p, (cos|sin)*lk]
    ]
    t = {name: nc.dram_tensor(name, shape, dt, kind="ExternalInput")
         for name, shape, dt in dt_in}
    yT = nc.dram_tensor("yT", [F, LQ], F32, kind="ExternalOutput")

    with tile.TileContext(nc) as tc:
        with (
            tc.tile_pool(name="persist", bufs=1) as persist,
            tc.tile_pool(name="ptiles", bufs=4) as ptp,
            tc.tile_pool(name="small", bufs=4) as small,
            tc.tile_pool(name="stgp", bufs=4) as stgp,
            tc.tile_pool(name="rbsp", bufs=4) as rbsp,
            tc.tile_pool(name="qrp", bufs=2) as qrp,
            tc.tile_pool(name="rtp", bufs=3) as rtp,
            tc.tile_pool(name="psacc", bufs=4, space="PSUM") as psacc,
            tc.tile_pool(name="psst", bufs=2, space="PSUM") as psst,
        ):
            qrot = persist.tile([128, PAIRS, LQ], F16)    # rotated qT
            ktops = persist.tile([128, LK], F16)  # kT in rows 0:64 AND 64:128
            vaug = persist.tile([128, KCH, D + 1], F16)   # V chunks + ones col
            obig = persist.tile([128, PAIRS, LQ], F16)    # O^T (unnorm->norm)
            # denominators partition-scattered: pairs 0..5 -> rows 0..11,
            # pairs 6..7 -> rows 32..35 (32-aligned engine bases for the two
            # batched reciprocals)
            densp = persist.tile([36, LQ], F16)
            rec32 = persist.tile([36, LQ], F32)
            recrow = persist.tile([1, 2 * PAIRS, LQ], F32)  # partition 0
            onesr = persist.tile([1, 128], F32)
            nc.vector.memset(onesr, 1.0)
            mt = persist.tile([128, KCH, LQ], F16)        # maskT resident

            # ---- small consts first on the sync queue ----
            bqbo = small.tile([128, 2 * FCH], F32, tag="bias")
            nc.sync.dma_start(bqbo, t["bqbo"].ap())
            bq_sb = bqbo[:, 0:FCH]
            bo_sb = bqbo[:, FCH:2 * FCH]
            bkv_sb = small.tile([128, 1], F32, tag="bias2")
            nc.sync.dma_start(bkv_sb, t["bkv"].ap().unsqueeze(1))
            cq = persist.tile([128, LQ], F16)
            sq = persist.tile([128, LQ], F16)
            cksk = persist.tile([D, 2, LK], F16)
            nc.sync.dma_start(cq, t["cosq"].ap())
            nc.sync.dma_start(sq, t["sinq"].ap())
            nc.sync.dma_start(cksk,
                              t["cksk"].ap().rearrange("p (a l) -> p a l", a=2))
            ck = cksk[:, 0, :]
            sk = cksk[:, 1, :]

            # ---- all inputs on the sync queue in dependency order (one
            # queue => transfers run in this order on the SDMA engines) ----
            wkv_sb = persist.tile([128, FCH, 128], F16)
            nc.sync.dma_start(wkv_sb, t["wkv"].ap())
            xkvt = []
            for l in range(NL):
                xkv = persist.tile([128, FCH, LQ], F16, tag=f"xkv{l}")
                nc.sync.dma_start(xkv, t["xkv_t"].ap()[l])
                xkvt.append(xkv)
            xq = persist.tile([128, FCH, LQ], F16)
            nc.sync.dma_start(xq, t["xq_t"].ap())
            wqall = persist.tile([128, FCH, FCH, 128], F16)
            nc.sync.dma_start(wqall, t["wq"].ap())
            nc.sync.dma_start(mt, t["mask_t"].ap())
            woall = persist.tile([128, FCH, FCH, 128], F16)
            nc.sync.dma_start(woall, t["wo"].ap())

            idtf = small.tile([128, 128], F32, tag="identf")
            make_identity(nc, idtf)
            idt = small.tile([128, 128], F16, tag="ident")
            nc.vector.tensor_copy(idt, idtf)
            # halves-swap permutation matrix: M[p, p-xor-32-within-head] = 1
            swpf = small.tile([128, 128], F32, tag="swpf")
            nc.gpsimd.memset(swpf, 0.0)
            for o1, o2 in ((0, 32), (32, 0), (64, 96), (96, 64)):
                nc.gpsimd.affine_select(
                    out=swpf[o1:o1 + 32, o2:o2 + 32],
                    in_=swpf[o1:o1 + 32, o2:o2 + 32],
                    compare_op=mybir.AluOpType.not_equal, fill=1.0,
                    base=0, pattern=[[-1, 32]], channel_multiplier=1)
            swp = small.tile([128, 128], F16, tag="swp")
            nc.vector.tensor_copy(swp, swpf)

            # ======== phase A: kv projection + k-RoPE ========
            kvraw = persist.tile([128, LK], F16)
            with tc.tile_pool(name="ktmp", bufs=1) as ktp:
                for l in range(NL):
                    pkv = psacc.tile([128, LQ], F32, tag="acc")
                    for f in range(FCH):
                        nc.tensor.matmul(pkv, wkv_sb[:, f, :],
                                         xkvt[l][:, f, :],
                                         start=(f == 0), stop=(f == FCH - 1))
                    nc.vector.tensor_scalar_add(kvraw[:, ts(l, LQ)], pkv,
                                                bkv_sb[:, 0:1])

                tmk = ktp.tile([D, LK], F16, tag="ksin")
                nc.vector.tensor_mul(tmk, kvraw[0:64], sk)
                kc = ktp.tile([D, LK], F16, tag="kcos")
                nc.vector.tensor_mul(kc, kvraw[0:64], ck)
                for l in range(NL):
                    pswk = psacc.tile([128, LQ], F32, tag="acc")
                    nc.tensor.matmul(pswk[0:64], swp[0:64, 0:64],
                                     tmk[:, ts(l, LQ)], start=True, stop=True)
                    nc.vector.tensor_add(ktops[0:64, ts(l, LQ)],
                                         kc[:, ts(l, LQ)], pswk[0:64])
                nc.gpsimd.dma_start(ktops[64:128], ktops[0:64])

            def qproj_rope(j):
                psq = psacc.tile([128, LQ], F32, tag="acc")
                for f in range(FCH):
                    nc.tensor.matmul(psq, wqall[:, j, f, :], xq[:, f, :],
                                     start=(f == 0), stop=(f == FCH - 1))
                qraw = qrp.tile([128, LQ], F16, tag="qraw")
                nc.vector.tensor_scalar_add(qraw, psq, bq_sb[:, j:j + 1])
                tmq = rtp.tile([128, LQ], F16, tag="qsin")
                nc.vector.tensor_mul(tmq, qraw, sq)
                psw = psacc.tile([128, LQ], F32, tag="acc")
                nc.tensor.matmul(psw, swp, tmq, start=True, stop=True)
                qc = rtp.tile([128, LQ], F16, tag="qcos")
                nc.vector.tensor_mul(qc, qraw, cq)
                nc.vector.tensor_add(qrot[:, j, :], qc, psw)

            # ========== phase C: attention (q-projection interleaved) =======
            for j in range(PAIRS):
                qproj_rope(j)
                if j == 0:
                    # V_aug: transpose v chunks, append ones column
                    nc.vector.memset(vaug[:, :, D:D + 1], 1.0)
                    for c in range(KCH):
                        tp = psacc.tile([128, D], F16, tag="acc")
                        nc.tensor.transpose(tp, kvraw[64:128, ts(c, 128)],
                                            idt[64:128, 64:128])
                        nc.vector.tensor_copy(vaug[:, c, 0:D], tp)

                oa = psacc.tile([128, LQ], F32, tag="acc")
                ob = psacc.tile([128, LQ], F32, tag="acc")
                for c in range(KCH):
                    if j == PAIRS - 2 and c == 5:
                        # batched reciprocal for pairs 0..5, placed mid-pair
                        # so the DVE-stream bubble is absorbed by the chunk
                        # pipeline's slack instead of stalling the PE
                        nc.vector.reciprocal(rec32[0:12, :], densp[0:12, :])
                        nc.sync.dma_start(recrow[0:1, 0:12, :], rec32[0:12, :])
                    st = psst.tile([128, 2, LQ], F32, tag="st")
                    # two concurrent K=64 row-tiled matmuls (heads 2j, 2j+1)
                    nc.tensor.matmul(st[:, 0, :], ktops[0:64, ts(c, 128)],
                                     qrot[0:64, j, :], start=True, stop=True)
                    nc.tensor.matmul(st[:, 1, :], ktops[64:128, ts(c, 128)],
                                     qrot[64:128, j, :], start=True, stop=True)
                    pt = ptp.tile([128, 2, LQ], F16, tag="p")
                    nc.scalar.activation(pt, st,
                                         mybir.ActivationFunctionType.Exp)
                    # one masked multiply for both heads (mask broadcast)
                    mb = mt[:, c, :].unsqueeze(1).broadcast_to((128, 2, LQ))
                    nc.vector.tensor_mul(pt, pt, mb)
                    nc.tensor.matmul(oa[0:D + 1, :], vaug[:, c, :],
                                     pt[:, 0, :], start=(c == 0),
                                     stop=(c == KCH - 1))
                    nc.tensor.matmul(ob[0:D + 1, :], vaug[:, c, :],
                                     pt[:, 1, :], start=(c == 0),
                                     stop=(c == KCH - 1))
                # evacuate unnormalized O^T + denominator row
                astg = stgp.tile([128, LQ], F16, tag="stg")
                nc.vector.tensor_copy(astg[0:D + 1], oa[0:D + 1])
                bstg = stgp.tile([128, LQ], F16, tag="stg")
                nc.vector.tensor_copy(bstg[0:D + 1], ob[0:D + 1])
                dp = 2 * j if j < PAIRS - 2 else 32 + 2 * (j - (PAIRS - 2))
                nc.sync.dma_start(obig[0:64, j, :], astg[0:64])
                nc.sync.dma_start(densp[dp:dp + 1, :], astg[64:65])
                nc.gpsimd.dma_start(obig[64:128, j, :], bstg[0:64])
                nc.gpsimd.dma_start(densp[dp + 1:dp + 2, :], bstg[64:65])

            # ======== normalization (all after the attention loop, so no
            # slow-dependency op ever precedes attention work in an engine's
            # static stream) ========
            def norm_pair(nj):
                ri = 2 * nj
                rbp = psacc.tile([128, LQ], F32, tag="acc")
                nc.tensor.matmul(rbp[0:64, :], onesr[0:1, 0:64],
                                 recrow[0:1, ri, :], start=True, stop=True)
                nc.tensor.matmul(rbp[64:128, :], onesr[0:1, 64:128],
                                 recrow[0:1, ri + 1, :], start=True, stop=True)
                rbs = rbsp.tile([128, LQ], F16, tag="rbs")
                nc.scalar.copy(rbs, rbp)
                nc.vector.tensor_mul(obig[:, nj, :], obig[:, nj, :], rbs)

            for nj in range(PAIRS - 2):
                norm_pair(nj)
            # pairs 6,7: reciprocal + gather, then normalize
            nc.vector.reciprocal(rec32[32:36, :], densp[32:36, :])
            nc.sync.dma_start(recrow[0:1, 12:16, :], rec32[32:36, :])
            norm_pair(PAIRS - 2)
            norm_pair(PAIRS - 1)

            # ================= phase D: output projection =================
            # two groups of 4 fb-chains; accumulate j=0..5 first so the
            # group's early matmuls overlap the pairs-6/7 normalization
            with tc.tile_pool(name="yout", bufs=2) as yout:
                for g in range(2):
                    fbs = range(4 * g, 4 * g + 4)
                    psys = {}
                    for fb in fbs:
                        psy_fb = psacc.tile([128, LQ], F32, tag="acc")
                        psys[fb] = psy_fb
                    for j in range(FCH):
                        for fb in fbs:
                            nc.tensor.matmul(psys[fb], woall[:, fb, j, :],
                                             obig[:, j, :],
                                             start=(j == 0),
                                             stop=(j == FCH - 1))
                    for fb in fbs:
                        ysb = yout.tile([128, LQ], F32, tag="y")
                        nc.vector.tensor_scalar_add(ysb, psys[fb],
                                                    bo_sb[:, fb:fb + 1])
                        nc.sync.dma_start(yT.ap()[ts(fb, 128), :], ysb)

    nc.compile()
    return nc


def _tables():
    """RoPE tables in halves-permuted basis: rows i (even-half) hold +sin,
    rows 32+i (odd-half) hold -sin (for the tmp-then-swap formulation)."""
    inv_freq = 1.0 / (10000.0 ** (np.arange(0, D, 2, dtype=np.float64) / D))
    ang = np.outer(inv_freq, np.arange(L, dtype=np.float64))  # [32, L]
    cos = np.cos(ang).astype(np.float32)
    sin = np.sin(ang).astype(np.float32)
    cos64 = np.concatenate([cos, cos], axis=0)                # [64, L]
    sin_sgn = np.concatenate([sin, -sin], axis=0)             # [64, L]
    return cos64, sin_sgn


def _pmajor(a, nch):
    """[nch*128, cols] -> [128, nch, cols] p-major contiguous."""
    return np.ascontiguousarray(
        a.reshape(nch, 128, -1).transpose(1, 0, 2)).astype(np.float16)


def _prep_weights(Wq, bq, Wk, bk, Wv, bv, Wo, bo):
    perm = np.concatenate([np.arange(0, D, 2), np.arange(1, D, 2)])
    WqP = np.asarray(Wq, dtype=np.float32)[:, :, perm].reshape(F, H * D)
    bqP = np.asarray(bq, dtype=np.float32)[:, perm].reshape(H * D)
    WkP = np.asarray(Wk, dtype=np.float32)[:, perm]
    bkP = np.asarray(bk, dtype=np.float32)[perm]
    Wkv = np.concatenate([WkP, np.asarray(Wv, dtype=np.float32)], axis=1)
    bkv = np.concatenate([bkP, np.asarray(bv, dtype=np.float32)])
    WoR = np.asarray(Wo, dtype=np.float32).reshape(H * D, F)
    bo_ = np.asarray(bo, dtype=np.float32)

    # wq[p, j, f, m] = WqP[f*128+p, j*128+m]
    wq_pret = np.ascontiguousarray(
        WqP.reshape(FCH, 128, FCH, 128).transpose(1, 2, 0, 3)).astype(
            np.float16)
    # wkv[p, f, m] = Wkv[f*128+p, m]
    wkv_pret = np.ascontiguousarray(
        Wkv.reshape(FCH, 128, 128).transpose(1, 0, 2)).astype(np.float16)
    # wo[p, fb, j, m] = WoR[j*128+p, fb*128+m]
    wo_pret = np.ascontiguousarray(
        WoR.reshape(FCH, 128, FCH, 128).transpose(1, 2, 0, 3)).astype(
            np.float16)
    bqbo = np.ascontiguousarray(np.concatenate(
        [bqP.reshape(FCH, 128).T, bo_.reshape(FCH, 128).T], axis=1))
    return wq_pret, wkv_pret, wo_pret, bqbo, bkv


def kernel(inputs_q, inputs_kv, mask, Wq, bq, Wk, bk, Wv, bv, Wo, bo):
    if "nc" not in _CACHED:
        _CACHED["nc"] = build_nc()
    nc = _CACHED["nc"]

    wq_pret, wkv_pret, wo_pret, bqbo, bkv = _prep_weights(
        Wq, bq, Wk, bk, Wv, bv, Wo, bo)

    cos64, sin_sgn = _tables()
    scale = 1.0 / np.sqrt(np.float32(D))
    cksk = np.ascontiguousarray(
        np.concatenate([cos64, sin_sgn], axis=1)).astype(np.float16)
    cosq_full = np.tile(cos64 * scale, (2, 1)).astype(np.float16)  # [128, L]
    sinq_full = np.tile(sin_sgn * scale, (2, 1)).astype(np.float16)

    xq = np.asarray(inputs_q, dtype=np.float32)
    xkv = np.asarray(inputs_kv, dtype=np.float32)
    mk = np.asarray(mask)

    in_maps = []
    for core in range(NCORES):
        b = core // 4
        qs = (core % 4) * LQ
        xq_t = _pmajor(xq[b, qs:qs + LQ, :].T, FCH)           # [128, f, lq]
        xkv_t = np.stack([
            _pmajor(xkv[b, l * LQ:(l + 1) * LQ, :].T, FCH)
            for l in range(NL)])                              # [l, 128, f, lq]
        mask_t = _pmajor(
            mk[b, 0, qs:qs + LQ, :].T.astype(np.float16), KCH)
        in_maps.append({
            "xq_t": xq_t,
            "xkv_t": xkv_t,
            "mask_t": mask_t,
            "wq": wq_pret,
            "wkv": wkv_pret,
            "wo": wo_pret,
            "bqbo": bqbo,
            "bkv": bkv,
            "cosq": np.ascontiguousarray(cosq_full[:, qs:qs + LQ]),
            "sinq": np.ascontiguousarray(sinq_full[:, qs:qs + LQ]),
            "cksk": cksk,
        })

    res = bass_utils.run_bass_kernel_spmd(nc, in_maps,
                                          core_ids=list(range(NCORES)))
    _CACHED["last_results"] = res
    _CACHED["last_maps"] = in_maps

    out = np.empty((B, L, F), dtype=np.float32)
    for core in range(NCORES):
        b = core // 4
        qs = (core % 4) * LQ
        out[b, qs:qs + LQ, :] = res.results[core]["yT"].T
    return out
